# revision 1
# baseline (speedup 1.0000x reference)
"""Trainium2 Bass kernel for a pre-norm causal-attention transformer layer.

Contract: kernel(**inputs) takes the FULL fp32 inputs of reference.setup_inputs()
and returns the FULL (1, 4096, 1024) fp32 output, distributing across 8
NeuronCores internally (heads tensor-parallel for attention, tokens
data-parallel for the output projection + FFN, one AllToAll in between).

Math notes (validated against the reference in fp64/numpy):
- LayerNorm gains are folded into the following weight matrices on the host:
  h @ W = z @ (g*W) + (ln_b @ W), where z = (x - mu) * rsig.
- The k-projection bias is dropped (softmax is shift-invariant along keys);
  the v bias is applied after normalization; the q bias rides the eviction.
- Softmax runs without max-subtraction (scores are bounded, |s| < ~3).
- Scores are built transposed (keys on partitions) so exp output feeds the
  PE directly; an appended ones-column of v yields the denominator row.
"""

import sys

sys.path.insert(0, "/opt/trn_rl_repo")

import ml_dtypes
import numpy as np

import concourse.bass as bass
from concourse import bacc, mybir, tile
from concourse.bass_utils import run_bass_kernel_spmd

F32 = mybir.dt.float32
BF = mybir.dt.bfloat16
bf16 = ml_dtypes.bfloat16

P = 128
E = 1024
NH = 16
HS = 64
D = 1024
FF = 4096
NCORES = 8
HPC = NH // NCORES  # heads per core = 2
LN_EPS = 1e-5
SCL = 1.0 / 32.0  # 1/sqrt(E)

Act = mybir.ActivationFunctionType
Alu = mybir.AluOpType


def _build(C):
    NT = C // P  # x tiles (32)
    NQC = C // 512  # q chunks (8)
    TPC = C // NCORES  # tokens per core (512)
    TT = TPC // P  # token tiles per core slice (4)
    NZG = max(1, NT // 8)  # zT groups of 8 x-tiles
    GL = NT // NZG  # x-tiles per zT group
    KT_E = E // P  # contraction tiles over E (8)
    KT_F = FF // P  # contraction tiles over FF (32)
    NFT = FF // P  # f tiles (32)

    nc = bacc.Bacc("TRN2", target_bir_lowering=False, debug=False, num_devices=NCORES)

    x_d = nc.dram_tensor("x", [C, E], BF, kind="ExternalInput")
    xs_d = nc.dram_tensor("xs", [TPC, E], F32, kind="ExternalInput")
    wq_d = nc.dram_tensor("wq", [KT_E, P, P], BF, kind="ExternalInput")
    wk_d = nc.dram_tensor("wk", [KT_E, P, P], BF, kind="ExternalInput")
    wv_d = nc.dram_tensor("wv", [KT_E, P, P], BF, kind="ExternalInput")
    bq_d = nc.dram_tensor("bq", [P, 1], F32, kind="ExternalInput")
    bv_d = nc.dram_tensor("bv", [P, 1], F32, kind="ExternalInput")
    wo_d = nc.dram_tensor("wo", [KT_E, P, D], BF, kind="ExternalInput")
    bo_d = nc.dram_tensor("bo_r", [1, D], BF, kind="ExternalInput")
    w1_d = nc.dram_tensor("w1", [KT_E, P, FF], BF, kind="ExternalInput")
    b1_d = nc.dram_tensor("b1c", [P, NFT], F32, kind="ExternalInput")
    w2_d = nc.dram_tensor("w2", [2, KT_F, P, E // 2], BF, kind="ExternalInput")
    b2_d = nc.dram_tensor("b2_r", [1, E], BF, kind="ExternalInput")
    mask_d = nc.dram_tensor("mask", [P, P], BF, kind="ExternalInput")
    maskz_d = nc.dram_tensor("maskz", [4, P, 512], BF, kind="ExternalInput")
    y_d = nc.dram_tensor("y", [TPC, E], F32, kind="ExternalOutput")
    y_view = y_d.ap().rearrange("(tc p) e -> p tc e", p=P)
    xs_view = xs_d.ap().rearrange("(tc p) e -> p tc e", p=P)

    with tile.TileContext(nc) as tc:
        with (
            tc.tile_pool(name="consts", bufs=1) as consts,
            tc.tile_pool(name="dram", bufs=1, space="DRAM") as dram,
        ):
            # ---- constants / weights resident in SBUF (~23 KB/part) ----
            wq_sb = consts.tile([P, KT_E, P], BF, tag="wq")
            wk_sb = consts.tile([P, KT_E, P], BF, tag="wk")
            wv_sb = consts.tile([P, KT_E, P], BF, tag="wv")
            nc.sync.dma_start(wq_sb, wq_d.ap().rearrange("k p m -> p k m"))
            nc.sync.dma_start(wk_sb, wk_d.ap().rearrange("k p m -> p k m"))
            nc.sync.dma_start(wv_sb, wv_d.ap().rearrange("k p m -> p k m"))
            wo_sb = consts.tile([P, KT_E, D], BF, tag="wo")
            nc.sync.dma_start(wo_sb, wo_d.ap().rearrange("k p n -> p k n"))
            bq_sb = consts.tile([P, 1], F32, tag="bq")
            bv_sb = consts.tile([P, 1], F32, tag="bv")
            nc.sync.dma_start(bq_sb, bq_d.ap())
            nc.sync.dma_start(bv_sb, bv_d.ap())
            bo_sb = consts.tile([1, D], BF, tag="bo")
            b2_sb = consts.tile([1, E], BF, tag="b2")
            nc.sync.dma_start(bo_sb, bo_d.ap())
            nc.sync.dma_start(b2_sb, b2_d.ap())
            b1_sb = consts.tile([P, NFT], F32, tag="b1")
            nc.sync.dma_start(b1_sb, b1_d.ap())
            mask_sb = consts.tile([P, P], BF, tag="mask")
            nc.sync.dma_start(mask_sb, mask_d.ap())
            maskz_sb = consts.tile([P, 4, 512], BF, tag="maskz")
            nc.sync.dma_start(maskz_sb, maskz_d.ap().rearrange("d p t -> p d t"))
            eps_sb = consts.tile([P, 1], F32, tag="eps")
            nc.vector.memset(eps_sb, LN_EPS)
            # bf16: fp32 matmul operands decompose into LOW/HIGH pairs
            ones_sb = consts.tile([1, P], BF, tag="ones")
            nc.vector.memset(ones_sb, 1.0)

            a2a_in_h = [dram.tile([NCORES, HS, TPC], BF, name=f"a2ai{h}", tag=f"a2ai{h}")
                        for h in range(HPC)]
            a2a_out_h = [dram.tile([NCORES, HS, TPC], BF, name=f"a2ao{h}", tag=f"a2ao{h}")
                         for h in range(HPC)]

            # ======== attention scope: qT/kT/v/outT (~32 KB/part) ========
            # chunked tiles (one per 512-token chunk) let attention start on
            # early chunks while q/k/v projections still run on later ones
            with tc.tile_pool(name="attnb", bufs=1) as attnb:
                NCH = C // 512
                # [128 partitions = 2 heads x 64 dims, 512 tokens] — scores
                # contract 64-deep at partition offset h*64 (no duplication)
                qT_c = [attnb.tile([P, 512], BF, name=f"qT{c}", tag=f"qT{c}")
                        for c in range(NCH)]
                kT_c = [attnb.tile([P, 512], BF, name=f"kT{c}", tag=f"kT{c}")
                        for c in range(NCH)]
                v_c = [attnb.tile([P, 4, HPC, HS + 1], BF, name=f"v{c}", tag=f"v{c}")
                       for c in range(NCH)]
                outT_c = [attnb.tile([P, 512], BF, name=f"oc{c}", tag=f"oc{c}")
                          for c in range(NCH)]
                for c in range(NCH):
                    nc.vector.memset(v_c[c][:, :, :, HS : HS + 1], 1.0)

                # ---- phase 1: LN1 + transpose (z with E on partitions) ----
                with (
                    tc.tile_pool(name="xp", bufs=GL + 2) as xp,
                    tc.tile_pool(name="zp", bufs=3) as zp,
                    tc.tile_pool(name="stp", bufs=3) as stp,
                    tc.tile_pool(name="ztp", bufs=1) as ztp,
                    tc.tile_pool(name="qkps", bufs=1, space="PSUM") as qkps,
                    tc.tile_pool(name="vps", bufs=1, space="PSUM") as vps,
                    tc.tile_pool(name="stps", bufs=2, space="PSUM") as stps,
                    tc.tile_pool(name="avps", bufs=2, space="PSUM") as avps,
                    tc.tile_pool(name="ep", bufs=8) as ep,
                    tc.tile_pool(name="nrm", bufs=4) as nrm,
                    tc.tile_pool(name="recd", bufs=4, space="DRAM") as recd,
                ):
                    zT_g = [
                        ztp.tile([P, GL, KT_E, P], BF, name=f"zT{g}", tag=f"zT{g}")
                        for g in range(NZG)
                    ]
                    # stats for a whole group of GL tiles first, then ONE
                    # batched Sqrt per group — the scalar engine otherwise
                    # thrashes activation tables (~2.6us/swap) alternating
                    # between LN-Sqrt and attention-Exp
                    for g in range(NZG):
                        # group 0 batches its Sqrt in chunk-size halves so the
                        # first attention chunk isn't gated on 8 tiles' stats
                        sub = 2 if g == 0 and GL >= 8 else 1
                        for sb_i in range(sub):
                            js = range(sb_i * GL // sub, (sb_i + 1) * GL // sub)
                            mv8 = stp.tile([P, GL, 2], F32, tag=f"mv8_{sb_i}")
                            x_gs = {}
                            for j in js:
                                t = g * GL + j
                                x_sb = xp.tile([P, E], BF, tag="xt")
                                nc.sync.dma_start(x_sb, x_d[t * P : (t + 1) * P, :])
                                x_gs[j] = x_sb
                                st = stp.tile([P, 2, 6], F32, tag="st")
                                nc.vector.bn_stats(st[:, 0, :], x_sb[:, 0:512])
                                nc.vector.bn_stats(st[:, 1, :], x_sb[:, 512:1024])
                                nc.vector.bn_aggr(mv8[:, j, :], st)
                            sig8 = stp.tile([P, GL], F32, tag=f"sig8_{sb_i}")
                            nc.scalar.activation(
                                sig8[:, js[0] : js[-1] + 1],
                                mv8[:, js[0] : js[-1] + 1, 1],
                                Act.Sqrt, bias=eps_sb, scale=1.0,
                            )
                            rs8 = stp.tile([P, GL], F32, tag=f"rs8_{sb_i}")
                            nc.vector.reciprocal(
                                rs8[:, js[0] : js[-1] + 1],
                                sig8[:, js[0] : js[-1] + 1],
                            )
                            for j in js:
                                z_sb = zp.tile([P, E], BF, tag="zt")
                                nc.vector.tensor_scalar(
                                    z_sb, x_gs[j], mv8[:, j, 0:1], rs8[:, j : j + 1],
                                    Alu.subtract, Alu.mult,
                                )
                                nc.sync.dma_start(
                                    zT_g[g][:, j, :, :], z_sb, transpose=True
                                )

                    # ---- phase 2+3 interleaved: each chunk's q/k/v is
                    # followed immediately by that chunk's head-0 attention so
                    # the exp chain starts ~200us earlier (the scheduler
                    # otherwise drains all projections first) ----
                    def qkv_chunk(c):
                        g, cl = (c * 4) // GL, (c * 4) % GL
                        rhs = zT_g[g][:, cl : cl + 4, :, :]
                        for nm in ("q", "k"):
                            w = wq_sb if nm == "q" else wk_sb
                            dst = qT_c[c] if nm == "q" else kT_c[c]
                            ps = qkps.tile([P, 512], F32, tag="qk")
                            for kt in range(KT_E):
                                nc.tensor.matmul(
                                    ps, w[:, kt, :], rhs[:, :, kt, :],
                                    start=(kt == 0), stop=(kt == KT_E - 1),
                                )
                            if nm == "q":
                                nc.vector.tensor_scalar(dst, ps, bq_sb, None, Alu.add)
                            else:
                                nc.vector.tensor_copy(dst, ps)
                        for tl in range(4):
                            t = c * 4 + tl
                            ps = vps.tile([P, P], F32, tag="vt")
                            for kt in range(KT_E):
                                nc.tensor.matmul(
                                    ps, zT_g[t // GL][:, t % GL, kt, :], wv_sb[:, kt, :],
                                    start=(kt == 0), stop=(kt == KT_E - 1),
                                )
                            nc.vector.tensor_copy(
                                v_c[c][:, tl, :, 0:HS],
                                ps.rearrange("p (h d) -> p h d", h=HPC),
                            )

                    def attn_chunk(h, qc):
                        hsl = slice(h * HS, (h + 1) * HS)
                        q_rhs = qT_c[qc][hsl, :]
                        av = avps.tile([HS + 1, 512], F32, tag="av")
                        nkb = 4 * qc + 4
                        for pr in range(nkb // 2):
                            sT = stps.tile([P, 1024], F32, tag="sT")
                            ex = ep.tile([P, 1024], BF, tag="ex")
                            for half in range(2):
                                kb = 2 * pr + half
                                nc.tensor.matmul(
                                    sT[:, half * 512 : half * 512 + 512],
                                    kT_c[kb // 4][hsl, (kb % 4) * P : (kb % 4 + 1) * P],
                                    q_rhs, start=True, stop=True,
                                )
                            nc.scalar.activation(ex, sT, Act.Exp, bias=0.0, scale=SCL)
                            for half in range(2):
                                kb = 2 * pr + half
                                dd = kb - 4 * qc
                                if dd >= 0:
                                    o = half * 512
                                    nc.vector.tensor_mul(
                                        ex[:, o : o + 512], ex[:, o : o + 512],
                                        maskz_sb[:, dd, :],
                                    )
                            for half in range(2):
                                kb = 2 * pr + half
                                nc.tensor.matmul(
                                    av, v_c[kb // 4][:, kb % 4, h, :],
                                    ex[:, half * 512 : half * 512 + 512],
                                    start=(kb == 0), stop=(kb == nkb - 1),
                                )
                        # normalize: denom row -> DRAM -> [128,4] scatter so the
                        # reciprocal runs 128 lanes wide, then broadcast back
                        avc = nrm.tile([HS + 1, 512], F32, tag="avc")
                        nc.vector.tensor_copy(avc, av)
                        rdr = recd.tile([1, 512], F32, tag="rdr")
                        nc.sync.dma_start(rdr, avc[HS : HS + 1, :])
                        rd = rdr[:]
                        scat = nrm.tile([P, 4], F32, tag="scat")
                        nc.sync.dma_start(
                            scat,
                            bass.AP(tensor=rd.tensor, offset=rd.offset,
                                    ap=[[4, P], [1, 4]]),
                        )
                        rec2 = nrm.tile([P, 4], F32, tag="rec2")
                        nc.vector.reciprocal(rec2, scat)
                        rdr2 = recd.tile([1, 512], F32, tag="rdr2")
                        rd2 = rdr2[:]
                        nc.sync.dma_start(
                            bass.AP(tensor=rd2.tensor, offset=rd2.offset,
                                    ap=[[4, P], [1, 4]]),
                            rec2,
                        )
                        bc = nrm.tile([HS, 512], F32, tag="bc")
                        nc.sync.dma_start(
                            bc,
                            bass.AP(tensor=rd2.tensor, offset=rd2.offset,
                                    ap=[[0, HS], rd2.ap[-1]]),
                        )
                        tmp = nrm.tile([HS, 512], F32, tag="tmp")
                        nc.gpsimd.tensor_mul(tmp, avc[0:HS, :], bc)
                        nc.vector.tensor_scalar(
                            outT_c[qc][hsl, :], tmp, bv_sb[hsl, 0:1], None, Alu.add
                        )
                        a2a_v3 = a2a_in_h[h][:].rearrange("j p t -> p j t")
                        if TPC >= 512:
                            nc.sync.dma_start(a2a_v3[:, qc, :], outT_c[qc][hsl, :])
                        else:
                            nj = 512 // TPC
                            nc.sync.dma_start(
                                a2a_v3[:, qc * nj : (qc + 1) * nj, :],
                                outT_c[qc][hsl, :].rearrange("p (j t) -> p j t", j=nj),
                            )

                    # head-1 trails head-0 by one chunk so the exp stream has
                    # work from the start; the last head-1 chunk runs after
                    # a2a0 and covers its transfer
                    for c in range(NCH):
                        qkv_chunk(c)
                        attn_chunk(0, c)
                        if c >= 1:
                            attn_chunk(1, c - 1)
                    nc.gpsimd.collective_compute(
                        "AllToAll", Alu.bypass,
                        replica_groups=[list(range(NCORES))],
                        ins=[a2a_in_h[0][:].opt()], outs=[a2a_out_h[0][:].opt()],
                    )
                    attn_chunk(1, NCH - 1)
                    nc.gpsimd.collective_compute(
                        "AllToAll", Alu.bypass,
                        replica_groups=[list(range(NCORES))],
                        ins=[a2a_in_h[1][:].opt()], outs=[a2a_out_h[1][:].opt()],
                    )

            # ======== FFN scope (attention buffers released) ========
            with tc.tile_pool(name="ffnb", bufs=1) as ffnb:
                x2_t = [ffnb.tile([P, E], F32, name=f"x2_{t}", tag=f"x2_{t}")
                        for t in range(TT)]
                fT_t = [ffnb.tile([P, TPC], BF, name=f"fT{f}", tag=f"fT{f}")
                        for f in range(NFT)]
                z2T_sb = ffnb.tile([P, TT, KT_E, P], BF, tag="z2T")

                with (
                    tc.tile_pool(name="w1p", bufs=1) as w1p,
                    tc.tile_pool(name="xsp", bufs=1) as xsp,
                    tc.tile_pool(name="st2p", bufs=2) as st2p,
                    tc.tile_pool(name="z2p", bufs=2) as z2p,
                    tc.tile_pool(name="wops", bufs=2, space="PSUM") as wops,
                    tc.tile_pool(name="mm1ps", bufs=2, space="PSUM") as mm1ps,
                    tc.tile_pool(name="mm2ps", bufs=1, space="PSUM") as mm2ps,
                    tc.tile_pool(name="w2p", bufs=8) as w2p,
                    tc.tile_pool(name="yout", bufs=3) as yout,
                ):
                    # a2a-independent loads issue first so they overlap the
                    # second AllToAll instead of queueing behind the oT load
                    xs_ts = [xsp.tile([P, E], F32, name=f"xs{t}", tag=f"xs{t}")
                             for t in range(TT)]
                    for t in range(TT):
                        nc.sync.dma_start(xs_ts[t], xs_view[:, t, :])
                    w1_sb = [
                        w1p.tile([P, FF], BF, name=f"w1_{k}", tag=f"w1_{k}")
                        for k in range(KT_E)
                    ]
                    for kt in range(KT_E):
                        nc.sync.dma_start(w1_sb[kt], w1_d[kt])

                    oT_sb = ffnb.tile([P, KT_E, TPC], BF, tag="oT")
                    nc.sync.dma_start(
                        oT_sb[0:HS, :, :], a2a_out_h[0][:].rearrange("j p t -> p j t")
                    )
                    nc.sync.dma_start(
                        oT_sb[HS:P, :, :], a2a_out_h[1][:].rearrange("j p t -> p j t")
                    )

                    # ---- phase 4: Wo projection + residual + LN2. Pass A
                    # contracts only the head-0 dims (available after a2a0) so
                    # the PE has work during the a2a1 wait; pass B adds the
                    # head-1 half, then stats -> one batched Sqrt, z2 on DVE ----
                    for t in range(TT):
                        xs_t = xs_ts[t]
                        for n in range(E // 512):
                            ns = slice(n * 512, (n + 1) * 512)
                            ps = wops.tile([P, 512], F32, tag="wo")
                            for kt in range(KT_E):
                                nc.tensor.matmul(
                                    ps, oT_sb[0:HS, kt, t * P : (t + 1) * P],
                                    wo_sb[0:HS, kt, ns],
                                    start=(kt == 0), stop=False,
                                )
                            nc.tensor.matmul(
                                ps, ones_sb, bo_sb[0:1, ns], start=False, stop=True
                            )
                            nc.vector.tensor_add(x2_t[t][:, ns], ps, xs_t[:, ns])
                    mv4 = st2p.tile([P, TT, 2], F32, tag="mv4")
                    for t in range(TT):
                        for n in range(E // 512):
                            ns = slice(n * 512, (n + 1) * 512)
                            ps = wops.tile([P, 512], F32, tag="wo")
                            for kt in range(KT_E):
                                nc.tensor.matmul(
                                    ps, oT_sb[HS:P, kt, t * P : (t + 1) * P],
                                    wo_sb[HS:P, kt, ns],
                                    start=(kt == 0), stop=(kt == KT_E - 1),
                                )
                            nc.vector.tensor_add(x2_t[t][:, ns], ps, x2_t[t][:, ns])

                        st = st2p.tile([P, 2, 6], F32, tag="st2")
                        nc.vector.bn_stats(st[:, 0, :], x2_t[t][:, 0:512])
                        nc.vector.bn_stats(st[:, 1, :], x2_t[t][:, 512:1024])
                        nc.vector.bn_aggr(mv4[:, t, :], st)
                        # LN2 sqrt in two half-batches so the first token
                        # tiles' z2T is ready before pass B finishes the rest
                        if t == TT // 2 - 1 or t == TT - 1:
                            ts0 = 0 if t < TT - 1 or TT < 2 else TT // 2
                            if TT < 2:
                                ts0 = 0
                            sig4 = st2p.tile([P, TT], F32, tag=f"sig4_{ts0}")
                            nc.scalar.activation(
                                sig4[:, ts0 : t + 1], mv4[:, ts0 : t + 1, 1],
                                Act.Sqrt, bias=eps_sb, scale=1.0,
                            )
                            rs4 = st2p.tile([P, TT], F32, tag=f"rs4_{ts0}")
                            nc.vector.reciprocal(
                                rs4[:, ts0 : t + 1], sig4[:, ts0 : t + 1]
                            )
                            for tz in range(ts0, t + 1):
                                z2 = z2p.tile([P, E], BF, tag="z2")
                                nc.vector.tensor_scalar(
                                    z2, x2_t[tz][:], mv4[:, tz, 0:1],
                                    rs4[:, tz : tz + 1],
                                    Alu.subtract, Alu.mult,
                                )
                                nc.sync.dma_start(
                                    z2T_sb[:, tz, :, :], z2, transpose=True
                                )

                    # ---- phase 5a: fT = relu(W1.T @ z2T + b1), in token
                    # halves — the first half only needs the first TT/2 z2T
                    # tiles and starts while pass B still runs on the rest ----
                    nhalf = 2 if TT >= 2 else 1
                    for half in range(nhalf):
                        tsl = slice(half * TT // nhalf, (half + 1) * TT // nhalf)
                        csl = slice(half * TPC // nhalf, (half + 1) * TPC // nhalf)
                        for ft in range(NFT):
                            ps = mm1ps.tile([P, TPC // nhalf], F32, tag="mm1")
                            for kt in range(KT_E):
                                nc.tensor.matmul(
                                    ps, w1_sb[kt][:, ft * P : (ft + 1) * P],
                                    z2T_sb[:, tsl, kt, :],
                                    start=(kt == 0), stop=(kt == KT_E - 1),
                                )
                            nc.scalar.activation(
                                fT_t[ft][:, csl], ps, Act.Relu,
                                bias=b1_sb[:, ft : ft + 1], scale=1.0,
                            )

                    # ---- phase 5b: y = fT.T @ W2 + b2 + x2, split by output
                    # column halves (W2 is streamed exactly once; <=4 psum
                    # banks per half so the first half interleaves with mm1) ----
                    for half in range(2):
                        ns = slice(half * 512, (half + 1) * 512)
                        ps2 = {
                            t: mm2ps.tile(
                                [P, 512], F32, name=f"y2_{half}_{t}",
                                tag=f"y2h_{t}",
                            )
                            for t in range(TT)
                        }
                        for kt in range(KT_F):
                            w2t = w2p.tile([P, 512], BF, tag="w2t")
                            nc.sync.dma_start(w2t, w2_d[half, kt])
                            for t in range(TT):
                                nc.tensor.matmul(
                                    ps2[t],
                                    fT_t[kt][:, t * P : (t + 1) * P],
                                    w2t,
                                    start=(kt == 0), stop=False,
                                )
                        for t in range(TT):
                            nc.tensor.matmul(
                                ps2[t], ones_sb, b2_sb[0:1, ns],
                                start=False, stop=True,
                            )
                            yt = yout.tile([P, 512], F32, tag="yt")
                            nc.vector.tensor_add(yt, ps2[t], x2_t[t][:, ns])
                            nc.sync.dma_start(y_view[:, t, ns], yt)

    nc.compile()
    return nc


_NC_CACHE = {}


def _get_nc(C):
    if C not in _NC_CACHE:
        _NC_CACHE[C] = _build(C)
    return _NC_CACHE[C]


def make_in_maps(inputs, C):
    """Host-side sharding + LN-gain folding. inputs values are numpy fp32."""
    TPC = C // NCORES
    KTE = E // P
    x32 = inputs["x"].reshape(C, E).astype(np.float32)
    x = np.ascontiguousarray(x32.astype(bf16))
    Wq, Wk, Wv = inputs["Wq"], inputs["Wk"], inputs["Wv"]
    Wo, bo = inputs["Wo"], inputs["bo"]
    W1, b1, W2, b2 = inputs["W1"], inputs["b1"], inputs["W2"], inputs["b2"]
    g1, bl1 = inputs["ln1_g"].astype(np.float64), inputs["ln1_b"].astype(np.float64)
    g2, bl2 = inputs["ln2_g"].astype(np.float64), inputs["ln2_b"].astype(np.float64)

    wo_h = np.ascontiguousarray(Wo.reshape(KTE, P, D).astype(bf16))
    w1_h = np.ascontiguousarray(
        (g2[:, None] * W1.astype(np.float64)).astype(np.float32)
        .reshape(KTE, P, FF).astype(bf16)
    )
    b1_eff = (b1.astype(np.float64) + bl2 @ W1.astype(np.float64)).astype(np.float32)
    b1c = np.ascontiguousarray(b1_eff.reshape(FF // P, P).T)  # (P, NFT)
    w2_h = np.ascontiguousarray(
        W2.reshape(FF // P, P, 2, E // 2).transpose(2, 0, 1, 3).astype(bf16)
    )
    b2r = np.ascontiguousarray(b2.reshape(1, E).astype(np.float32).astype(bf16))
    bor = np.ascontiguousarray(bo.reshape(1, D).astype(np.float32).astype(bf16))
    mask = np.ascontiguousarray(np.triu(np.ones((P, P), np.float32)).astype(bf16))
    tri = np.triu(np.ones((P, P), np.float32))
    maskz = np.zeros((4, P, 512), np.float32)
    for dd in range(4):
        maskz[dd, :, dd * P : (dd + 1) * P] = tri
        maskz[dd, :, (dd + 1) * P :] = 1.0
    maskz = np.ascontiguousarray(maskz.astype(bf16))

    in_maps = []
    for i in range(NCORES):
        h0, h1 = HPC * i, HPC * i + 1
        wq_eff = np.concatenate(
            [(g1[:, None] * Wq[h].astype(np.float64)) for h in (h0, h1)], axis=1
        ).astype(np.float32)  # (E, 128)
        wk_eff = np.concatenate(
            [(g1[:, None] * Wk[h].astype(np.float64)) for h in (h0, h1)], axis=1
        ).astype(np.float32)
        wv_eff = np.concatenate(
            [(g1[:, None] * Wv[h].astype(np.float64)) for h in (h0, h1)], axis=1
        ).astype(np.float32)
        bq = np.concatenate(
            [bl1 @ Wq[h].astype(np.float64) for h in (h0, h1)]
        ).astype(np.float32)
        bv = np.concatenate(
            [bl1 @ Wv[h].astype(np.float64) for h in (h0, h1)]
        ).astype(np.float32)
        in_maps.append(
            {
                "x": x,
                "xs": np.ascontiguousarray(x32[i * TPC : (i + 1) * TPC]),
                "wq": np.ascontiguousarray(wq_eff.reshape(KTE, P, P).astype(bf16)),
                "wk": np.ascontiguousarray(wk_eff.reshape(KTE, P, P).astype(bf16)),
                "wv": np.ascontiguousarray(wv_eff.reshape(KTE, P, P).astype(bf16)),
                "bq": np.ascontiguousarray(bq.reshape(P, 1)),
                "bv": np.ascontiguousarray(bv.reshape(P, 1)),
                "wo": wo_h,
                "bo_r": bor,
                "w1": w1_h,
                "b1c": b1c,
                "w2": w2_h,
                "b2_r": b2r,
                "mask": mask,
                "maskz": maskz,
            }
        )
    return in_maps


def run(inputs, C=4096, trace=False):
    nc = _get_nc(C)
    in_maps = make_in_maps(inputs, C)
    res = run_bass_kernel_spmd(nc, in_maps, core_ids=list(range(NCORES)), trace=trace)
    TPC = C // NCORES
    y = np.concatenate(
        [np.asarray(res.results[i]["y"]).reshape(TPC, E) for i in range(NCORES)], 0
    )
    return y.reshape(1, C, E).astype(np.float32), res


def kernel(**inputs):
    inputs = {k: np.asarray(v) for k, v in inputs.items()}
    y, _ = run(inputs, C=4096, trace=False)
    return y



# revision 10
# speedup vs baseline: 1.0978x; 1.0978x over previous
"""Trainium2 Bass kernel for a pre-norm causal-attention transformer layer.

Contract: kernel(**inputs) takes the FULL fp32 inputs of reference.setup_inputs()
and returns the FULL (1, 4096, 1024) fp32 output, distributing across 8
NeuronCores internally (heads tensor-parallel for attention, tokens
data-parallel for the output projection + FFN, two AllToAlls in between).

Design notes (v2):
- LayerNorm gains fold into the following weights on the host; the v-projection
  bias folds THROUGH Wo into bo (softmax weights sum to 1), so attention
  outputs travel unnormalized: the PV product plus a denominator row go
  through the AllToAll and are normalized after it (recip + K=8 broadcast
  matmuls), removing the whole pre-a2a normalization chain.
- Attention processes one 128-key block for BOTH local heads per event: the
  two score matmuls occupy disjoint PE row groups (partitions 0-63 / 64-127)
  so they overlap in the array, and a single [128,1024] Exp covers both.
- Emission is a software pipeline: LN1 and QKV matmul "thunks" are drip-fed
  between attention events so no engine queue stalls on a long dependency;
  score matmuls for event k+1 are emitted before PV matmuls of event k to
  hide the exp latency.
- Head-1's last three chunks run after the head-0 AllToAll is triggered
  (covering it); Wo pass A (head-0 half, K=64) covers the head-1 AllToAll.
- DMA issue queues: x/transposes + FFN weight loads + a2a staging on sync,
  a2a inputs + small consts on gpsimd, w2 stream on vector.
"""

import sys

sys.path.insert(0, "/opt/trn_rl_repo")

import ml_dtypes
import numpy as np

import concourse.bass as bass
from concourse import bacc, mybir, tile
from concourse.bass_utils import run_bass_kernel_spmd

F32 = mybir.dt.float32
BF = mybir.dt.bfloat16
bf16 = ml_dtypes.bfloat16

P = 128
E = 1024
NH = 16
HS = 64
D = 1024
FF = 4096
NCORES = 8
HPC = NH // NCORES  # heads per core = 2
LN_EPS = 1e-5
SCL = 1.0 / 32.0  # 1/sqrt(E)

Act = mybir.ActivationFunctionType
Alu = mybir.AluOpType


def _build(C):
    NT = C // P  # x tiles (32)
    NCH = C // 512  # chunks (8)
    TPC = C // NCORES  # tokens per core (512)
    TT = TPC // P  # token tiles per core slice (4)
    KT_E = E // P  # contraction tiles over E (8)
    KT_F = FF // P  # contraction tiles over FF (32)
    NFT = FF // P  # f tiles (32)
    SPL0 = NCH - 3 if NCH >= 6 else NCH  # head-1-deferred chunks start

    nc = bacc.Bacc("TRN2", target_bir_lowering=False, debug=False, num_devices=NCORES)

    x_d = nc.dram_tensor("x", [C, E], BF, kind="ExternalInput")
    xs_d = nc.dram_tensor("xs", [TPC, E], F32, kind="ExternalInput")
    wq_d = nc.dram_tensor("wq", [KT_E, P, P], BF, kind="ExternalInput")
    wk_d = nc.dram_tensor("wk", [KT_E, P, P], BF, kind="ExternalInput")
    wv_d = nc.dram_tensor("wv", [KT_E, P, P], BF, kind="ExternalInput")
    bq_d = nc.dram_tensor("bq", [P, 1], F32, kind="ExternalInput")
    wo_d = nc.dram_tensor("wo", [KT_E, P, D], BF, kind="ExternalInput")
    boe_d = nc.dram_tensor("boe", [1, D], BF, kind="ExternalInput")
    w1_d = nc.dram_tensor("w1", [KT_E, P, FF], BF, kind="ExternalInput")
    b1_d = nc.dram_tensor("b1c", [P, NFT], F32, kind="ExternalInput")
    w2_d = nc.dram_tensor("w2", [2, KT_F, P, E // 2], BF, kind="ExternalInput")
    b2_d = nc.dram_tensor("b2_r", [1, E], BF, kind="ExternalInput")
    maskz_d = nc.dram_tensor("maskz", [4, P, 512], BF, kind="ExternalInput")
    maskz2_d = nc.dram_tensor("maskz2", [4, P, 1024], BF, kind="ExternalInput")
    sel_d = nc.dram_tensor("sel", [2, KT_E, 8, HS], BF, kind="ExternalInput")
    y_d = nc.dram_tensor("y", [TPC, E], F32, kind="ExternalOutput")
    y_view = y_d.ap().rearrange("(tc p) e -> p tc e", p=P)
    xs_view = xs_d.ap().rearrange("(tc p) e -> p tc e", p=P)

    with tile.TileContext(nc) as tc:
        with (
            tc.tile_pool(name="consts", bufs=1) as consts,
            tc.tile_pool(name="dram", bufs=1, space="DRAM") as dram,
        ):
            # ---- small constants, loaded via the (otherwise idle) gpsimd q ----
            wq_sb = consts.tile([P, KT_E, P], BF, tag="wq")
            wk_sb = consts.tile([P, KT_E, P], BF, tag="wk")
            wv_sb = consts.tile([P, KT_E, P], BF, tag="wv")
            bq_sb = consts.tile([P, 1], F32, tag="bq")
            maskz_sb = consts.tile([P, 4, 512], BF, tag="maskz")
            maskz2_sb = consts.tile([P, 4, 1024], BF, tag="maskz2")
            sel_sb = consts.tile([8, 2, KT_E, HS], BF, tag="sel")
            b1_sb = consts.tile([P, NFT], F32, tag="b1")
            boe_sb = consts.tile([1, D], BF, tag="boe")
            b2_sb = consts.tile([1, E], BF, tag="b2")
            eps_sb = consts.tile([P, 1], F32, tag="eps")
            ones_sb = consts.tile([1, P], BF, tag="ones")
            nc.gpsimd.dma_start(wq_sb, wq_d.ap().rearrange("k p m -> p k m"))
            nc.gpsimd.dma_start(wk_sb, wk_d.ap().rearrange("k p m -> p k m"))
            nc.gpsimd.dma_start(wv_sb, wv_d.ap().rearrange("k p m -> p k m"))
            nc.gpsimd.dma_start(bq_sb, bq_d.ap())
            nc.gpsimd.dma_start(maskz_sb, maskz_d.ap().rearrange("d p t -> p d t"))
            nc.gpsimd.dma_start(maskz2_sb, maskz2_d.ap().rearrange("d p t -> p d t"))
            nc.gpsimd.dma_start(sel_sb, sel_d.ap().rearrange("h k r d -> r h k d"))
            nc.gpsimd.dma_start(b1_sb, b1_d.ap())
            nc.gpsimd.dma_start(boe_sb, boe_d.ap())
            nc.gpsimd.dma_start(b2_sb, b2_d.ap())
            nc.vector.memset(eps_sb, LN_EPS)
            nc.vector.memset(ones_sb, 1.0)

            a2a_in_h = [dram.tile([NCORES, HS + 1, TPC], BF, name=f"a2ai{h}",
                                  tag=f"a2ai{h}") for h in range(HPC)]
            a2a_out_h = [dram.tile([NCORES, HS + 1, TPC], BF, name=f"a2ao{h}",
                                   tag=f"a2ao{h}") for h in range(HPC)]

            with tc.tile_pool(name="attnb", bufs=1) as attnb:
                qT_c = [attnb.tile([P, 512], BF, name=f"qT{c}", tag=f"qT{c}")
                        for c in range(NCH)]
                kT_c = [attnb.tile([P, 512], BF, name=f"kT{c}", tag=f"kT{c}")
                        for c in range(NCH)]
                v_c = [attnb.tile([P, 4, HPC, HS + 1], BF, name=f"v{c}",
                                  tag=f"v{c}") for c in range(NCH)]
                for c in range(NCH):
                    nc.vector.memset(v_c[c][:, :, :, HS : HS + 1], 1.0)

                with (
                    tc.tile_pool(name="stps", bufs=2, space="PSUM") as stps,
                    tc.tile_pool(name="avps", bufs=1, space="PSUM") as avps,
                    tc.tile_pool(name="ep", bufs=6) as ep,
                    tc.tile_pool(name="outp", bufs=3) as outp,
                ):
                    # ---------- attention event engine ----------
                    pend_av = [None]
                    qkv_pend = []
                    ln_done = [0]
                    ln_thunks = []

                    def flush_av():
                        if pend_av[0] is not None:
                            pend_av[0]()
                            pend_av[0] = None

                    def qkv_drain(n):
                        while n > 0 and qkv_pend:
                            qkv_pend.pop(0)()
                            n -= 1

                    def ln_drain(n):
                        while n > 0 and ln_done[0] < len(ln_thunks):
                            ln_thunks[ln_done[0]]()
                            ln_done[0] += 1
                            n -= 1

                    def ev_step(emit_scores, emit_av):
                        emit_scores()
                        qkv_drain(2)
                        ln_drain(1)
                        flush_av()
                        pend_av[0] = emit_av

                    def merged_chunk(qc, av0, av1):
                        # both heads, one event per 128-key block
                        nkb = 4 * qc + 4
                        for kb in range(nkb):
                            sT = stps.tile([P, 1024], F32, tag="sT")
                            ex = ep.tile([P, 1024], BF, tag="ex")
                            kc, ko = kb // 4, (kb % 4) * P

                            def scores(sT=sT, ex=ex, kc=kc, ko=ko, kb=kb):
                                nc.tensor.matmul(
                                    sT[:, 0:512], kT_c[kc][0:HS, ko : ko + P],
                                    qT_c[qc][0:HS, :], start=True, stop=True)
                                nc.tensor.matmul(
                                    sT[:, 512:1024], kT_c[kc][HS:P, ko : ko + P],
                                    qT_c[qc][HS:P, :], start=True, stop=True)
                                nc.scalar.activation(ex, sT, Act.Exp,
                                                     bias=0.0, scale=SCL)
                                dd = kb - 4 * qc
                                if dd >= 0:
                                    nc.vector.tensor_mul(
                                        ex, ex, maskz2_sb[:, dd, :])

                            def av(ex=ex, kc=kc, kb=kb, first=(kb == 0),
                                   last=(kb == nkb - 1)):
                                nc.tensor.matmul(
                                    av0, v_c[kc][:, kb % 4, 0, :],
                                    ex[:, 0:512], start=first, stop=last)
                                nc.tensor.matmul(
                                    av1, v_c[kc][:, kb % 4, 1, :],
                                    ex[:, 512:1024], start=first, stop=last)

                            ev_step(scores, av)

                    def split_chunk(qc, h, avx):
                        # one head, one event per pair of key blocks
                        nkb = 4 * qc + 4
                        hsl = slice(h * HS, (h + 1) * HS)
                        for pr in range(nkb // 2):
                            sT = stps.tile([P, 1024], F32, tag="sT")
                            ex = ep.tile([P, 1024], BF, tag="ex")

                            def scores(sT=sT, ex=ex, pr=pr):
                                for half in range(2):
                                    kb = 2 * pr + half
                                    nc.tensor.matmul(
                                        sT[:, half * 512 : half * 512 + 512],
                                        kT_c[kb // 4][hsl,
                                                      (kb % 4) * P : (kb % 4 + 1) * P],
                                        qT_c[qc][hsl, :], start=True, stop=True)
                                nc.scalar.activation(ex, sT, Act.Exp,
                                                     bias=0.0, scale=SCL)
                                for half in range(2):
                                    dd = 2 * pr + half - 4 * qc
                                    if dd >= 0:
                                        o = half * 512
                                        nc.vector.tensor_mul(
                                            ex[:, o : o + 512],
                                            ex[:, o : o + 512],
                                            maskz_sb[:, dd, :])

                            def av(ex=ex, pr=pr, first=(pr == 0),
                                   last=(pr == nkb // 2 - 1)):
                                for half in range(2):
                                    kb = 2 * pr + half
                                    nc.tensor.matmul(
                                        avx, v_c[kb // 4][:, kb % 4, h, :],
                                        ex[:, half * 512 : half * 512 + 512],
                                        start=(first and half == 0),
                                        stop=(last and half == 1))

                            ev_step(scores, av)

                    def evict(avx, h, qc):
                        outT = outp.tile([HS + 1, 512], BF, tag="outT")
                        nc.vector.tensor_copy(outT, avx)
                        a2a_v3 = a2a_in_h[h][:].rearrange("j p t -> p j t")
                        nc.gpsimd.dma_start(a2a_v3[:, qc, :], outT)

                    # ---------- phase alpha: LN1 + qkv + attention ----------
                    with (
                        tc.tile_pool(name="xp", bufs=18) as xp,
                        tc.tile_pool(name="zp", bufs=3) as zp,
                        tc.tile_pool(name="stp", bufs=2) as stp,
                        tc.tile_pool(name="ztp", bufs=3) as ztp,
                        tc.tile_pool(name="qkps", bufs=1, space="PSUM") as qkps,
                        tc.tile_pool(name="vps", bufs=1, space="PSUM") as vps,
                    ):
                        BATCHES = [(0, 4), (4, 16), (16, NT)]
                        mv_sb = stp.tile([P, NT, 2], F32, tag="mv")
                        rs_sb = stp.tile([P, NT], F32, tag="rs")
                        x_tiles = {}
                        zT_c = [None] * NCH

                        def ln_load(t):
                            x_sb = xp.tile([P, E], BF, tag="xt")
                            nc.sync.dma_start(x_sb, x_d[t * P : (t + 1) * P, :])
                            x_tiles[t] = x_sb
                            st = stp.tile([P, 2, 6], F32, tag="st")
                            nc.vector.bn_stats(st[:, 0, :], x_sb[:, 0:512])
                            nc.vector.bn_stats(st[:, 1, :], x_sb[:, 512:1024])
                            nc.vector.bn_aggr(mv_sb[:, t, :], st)

                        def ln_sqrt(b0, b1):
                            sig = stp.tile([P, NT], F32, tag="sig")
                            nc.scalar.activation(
                                sig[:, b0:b1], mv_sb[:, b0:b1, 1],
                                Act.Sqrt, bias=eps_sb, scale=1.0)
                            nc.vector.reciprocal(rs_sb[:, b0:b1], sig[:, b0:b1])

                        def ln_z(t):
                            c, j = t // 4, t % 4
                            if zT_c[c] is None:
                                zT_c[c] = ztp.tile([P, 4, KT_E, P], BF,
                                                   name=f"zT{c}", tag="zT")
                            z_sb = zp.tile([P, E], BF, tag="zt")
                            nc.vector.tensor_scalar(
                                z_sb, x_tiles.pop(t), mv_sb[:, t, 0:1],
                                rs_sb[:, t : t + 1], Alu.subtract, Alu.mult)
                            nc.sync.dma_start(zT_c[c][:, j, :, :], z_sb,
                                              transpose=True)

                        for b0, b1 in BATCHES:
                            for t in range(b0, b1):
                                ln_thunks.append(lambda t=t: ln_load(t))
                            ln_thunks.append(lambda b0=b0, b1=b1: ln_sqrt(b0, b1))
                            for t in range(b0, b1):
                                ln_thunks.append(lambda t=t: ln_z(t))

                        def ln_need(c):
                            # drain through ln_z of chunk c's last tile
                            t_last = c * 4 + 3
                            cnt = 0
                            need = len(ln_thunks)
                            for b0, b1 in BATCHES:
                                cnt += (b1 - b0) + 1
                                if t_last < b1:
                                    need = cnt + (t_last - b0 + 1)
                                    break
                                cnt += b1 - b0
                            while ln_done[0] < need:
                                ln_drain(1)

                        def mk_qkv_thunks(c):
                            th = []
                            ps_ref = {}

                            def qk_mms(nm, i):
                                if i == 0:
                                    ps_ref[nm] = qkps.tile(
                                        [P, 512], F32, name=f"qk_{nm}", tag="qk")
                                ps = ps_ref[nm]
                                w = wq_sb if nm == "q" else wk_sb
                                for kt in (2 * i, 2 * i + 1):
                                    nc.tensor.matmul(
                                        ps, w[:, kt, :], zT_c[c][:, :, kt, :],
                                        start=(kt == 0), stop=(kt == KT_E - 1))
                                if i == 3:
                                    dst = qT_c[c] if nm == "q" else kT_c[c]
                                    if nm == "q":
                                        nc.vector.tensor_scalar(
                                            dst, ps, bq_sb, None, Alu.add)
                                    else:
                                        nc.vector.tensor_copy(dst, ps)

                            def v_mms(tl, i):
                                if tl == 0 and i == 0:
                                    ps_ref["v"] = vps.tile(
                                        [P, 512], F32, name="vpst", tag="v")
                                ps = ps_ref["v"]
                                for kt in (2 * i, 2 * i + 1):
                                    nc.tensor.matmul(
                                        ps[:, tl * P : (tl + 1) * P],
                                        zT_c[c][:, tl, kt, :], wv_sb[:, kt, :],
                                        start=(kt == 0), stop=(kt == KT_E - 1))
                                if i == 3:
                                    nc.vector.tensor_copy(
                                        v_c[c][:, tl, :, 0:HS],
                                        ps[:, tl * P : (tl + 1) * P].rearrange(
                                            "p (h d) -> p h d", h=HPC))

                            for nm in ("q", "k"):
                                for i in range(4):
                                    th.append(lambda nm=nm, i=i: qk_mms(nm, i))
                            for tl in range(4):
                                for i in range(4):
                                    th.append(lambda tl=tl, i=i: v_mms(tl, i))
                            return th

                        qkv_ready = [0]

                        def qkv_feed(c):
                            if qkv_ready[0] <= c:
                                ln_need(c)
                                qkv_pend.extend(mk_qkv_thunks(c))
                                qkv_ready[0] = c + 1

                        def qkv_need(c):
                            qkv_feed(c)
                            if qkv_ready[0] == c + 1:
                                qkv_drain(len(qkv_pend))

                        ln_drain(5)  # chunk-0 x loads + first sqrt under way
                        qkv_need(0)
                        qkv_feed(1)
                        for qc in range(NCH):
                            qkv_need(qc)
                            if qc + 1 < NCH:
                                qkv_feed(qc + 1)
                            if qc < SPL0:
                                av0 = avps.tile([HS + 1, 512], F32, tag="av0")
                                av1 = avps.tile([HS + 1, 512], F32, tag="av1")
                                merged_chunk(qc, av0, av1)
                                flush_av()
                                evict(av0, 0, qc)
                                evict(av1, 1, qc)
                            else:
                                av0 = avps.tile([HS + 1, 512], F32, tag="av0")
                                split_chunk(qc, 0, av0)
                                flush_av()
                                evict(av0, 0, qc)
                        ln_drain(len(ln_thunks))

                    # alpha-lifetime pools closed: head-0 a2a can go
                    nc.gpsimd.collective_compute(
                        "AllToAll", Alu.bypass,
                        replica_groups=[list(range(NCORES))],
                        ins=[a2a_in_h[0][:].opt()], outs=[a2a_out_h[0][:].opt()],
                    )

                    # ---------- phase beta: head-1 deferred chunks ----------
                    for qc in range(SPL0, NCH):
                        av1 = avps.tile([HS + 1, 512], F32, tag="av1")
                        split_chunk(qc, 1, av1)
                        flush_av()
                        evict(av1, 1, qc)

            # ======== Wo + LN2 + FFN scope (attention buffers released) ========
            with tc.tile_pool(name="ffw", bufs=1) as ffw:
                # staging + weight loads; transfers overlap a2a1
                oT_sb = ffw.tile([P, KT_E, TPC], BF, tag="oT")
                den_sb = ffw.tile([8, HPC, 512], BF, tag="den")
                rec_sb = ffw.tile([8, HPC, 512], BF, tag="rec")
                wo_sb = ffw.tile([P, KT_E, D], BF, tag="wo")
                x2_t = [ffw.tile([P, E], F32, name=f"x2_{t}", tag=f"x2_{t}")
                        for t in range(TT)]
                z2T_sb = ffw.tile([P, TT, KT_E, P], BF, tag="z2T")
                nc.sync.dma_start(
                    oT_sb[0:HS, :, :],
                    a2a_out_h[0][:, 0:HS, :].rearrange("j p t -> p j t"))
                nc.sync.dma_start(
                    den_sb[:, 0, :],
                    a2a_out_h[0][:, HS : HS + 1, :].rearrange("j o t -> (j o) t"))
                nc.sync.dma_start(wo_sb, wo_d.ap().rearrange("k p n -> p k n"))
                for t in range(TT):
                    nc.sync.dma_start(x2_t[t], xs_view[:, t, :])
                w1_sb = [ffw.tile([P, FF], BF, name=f"w1_{k}", tag=f"w1_{k}")
                         for k in range(KT_E)]
                for kt in range(KT_E):
                    nc.sync.dma_start(w1_sb[kt], w1_d[kt])

                # normalize head-0 in place (overlaps a2a1)
                with nc.allow_low_precision(reason="bf16 softmax denom recip"):
                    nc.vector.reciprocal(rec_sb[:, 0, :], den_sb[:, 0, :])
                with tc.tile_pool(name="bcps", bufs=2, space="PSUM") as bcps:
                    for kt in range(KT_E):
                        bc = bcps.tile([HS, 512], F32, tag="bc")
                        nc.tensor.matmul(bc, sel_sb[:, 0, kt, :],
                                         rec_sb[:, 0, :], start=True, stop=True)
                        nc.vector.tensor_mul(
                            oT_sb[0:HS, kt, :], oT_sb[0:HS, kt, :], bc)

                nc.gpsimd.collective_compute(
                    "AllToAll", Alu.bypass,
                    replica_groups=[list(range(NCORES))],
                    ins=[a2a_in_h[1][:].opt()], outs=[a2a_out_h[1][:].opt()],
                )

                with (
                    tc.tile_pool(name="st2p", bufs=2) as st2p,
                    tc.tile_pool(name="z2p", bufs=2) as z2p,
                    tc.tile_pool(name="wops", bufs=2, space="PSUM") as wops,
                    tc.tile_pool(name="bc2ps", bufs=2, space="PSUM") as bc2ps,
                ):
                    # ---- Wo pass A: head-0 halves (overlaps a2a1) ----
                    for t in range(TT):
                        for n in range(E // 512):
                            ns = slice(n * 512, (n + 1) * 512)
                            ps = wops.tile([P, 512], F32, tag="wo")
                            for kt in range(KT_E):
                                nc.tensor.matmul(
                                    ps, oT_sb[0:HS, kt, t * P : (t + 1) * P],
                                    wo_sb[0:HS, kt, ns],
                                    start=(kt == 0), stop=False)
                            nc.tensor.matmul(ps, ones_sb, boe_sb[0:1, ns],
                                             start=False, stop=True)
                            nc.vector.tensor_add(x2_t[t][:, ns], ps,
                                                 x2_t[t][:, ns])

                    # ---- head-1 staging + normalize (after a2a1) ----
                    nc.sync.dma_start(
                        oT_sb[HS:P, :, :],
                        a2a_out_h[1][:, 0:HS, :].rearrange("j p t -> p j t"))
                    nc.sync.dma_start(
                        den_sb[:, 1, :],
                        a2a_out_h[1][:, HS : HS + 1, :].rearrange("j o t -> (j o) t"))
                    with nc.allow_low_precision(
                            reason="bf16 softmax denom recip"):
                        nc.vector.reciprocal(rec_sb[:, 1, :], den_sb[:, 1, :])
                    for kt in range(KT_E):
                        bc = bc2ps.tile([HS, 512], F32, tag="bc2")
                        nc.tensor.matmul(bc, sel_sb[:, 1, kt, :],
                                         rec_sb[:, 1, :], start=True, stop=True)
                        nc.vector.tensor_mul(
                            oT_sb[HS:P, kt, :], oT_sb[HS:P, kt, :], bc)

                    # ---- Wo pass B + residual + LN2 ----
                    mv4 = st2p.tile([P, TT, 2], F32, tag="mv4")
                    for t in range(TT):
                        for n in range(E // 512):
                            ns = slice(n * 512, (n + 1) * 512)
                            ps = wops.tile([P, 512], F32, tag="wo")
                            for kt in range(KT_E):
                                nc.tensor.matmul(
                                    ps, oT_sb[HS:P, kt, t * P : (t + 1) * P],
                                    wo_sb[HS:P, kt, ns],
                                    start=(kt == 0), stop=(kt == KT_E - 1))
                            nc.vector.tensor_add(x2_t[t][:, ns], ps,
                                                 x2_t[t][:, ns])
                        st = st2p.tile([P, 2, 6], F32, tag="st2")
                        nc.vector.bn_stats(st[:, 0, :], x2_t[t][:, 0:512])
                        nc.vector.bn_stats(st[:, 1, :], x2_t[t][:, 512:1024])
                        nc.vector.bn_aggr(mv4[:, t, :], st)

                    sig4 = st2p.tile([P, TT], F32, tag="sig4")
                    nc.scalar.activation(sig4, mv4[:, :, 1], Act.Sqrt,
                                         bias=eps_sb, scale=1.0)
                    rs4 = st2p.tile([P, TT], F32, tag="rs4")
                    nc.vector.reciprocal(rs4, sig4)
                    for tz in range(TT):
                        z2 = z2p.tile([P, E], BF, tag="z2")
                        nc.vector.tensor_scalar(
                            z2, x2_t[tz][:], mv4[:, tz, 0:1],
                            rs4[:, tz : tz + 1], Alu.subtract, Alu.mult)
                        nc.sync.dma_start(z2T_sb[:, tz, :, :], z2, transpose=True)

                # ---- FFN: mm1 (relu) interleaved with mm2 half 0 ----
                fT_t = [ffw.tile([P, TPC], BF, name=f"fT{f}", tag=f"fT{f}")
                        for f in range(NFT)]
                with (
                    tc.tile_pool(name="mm1ps", bufs=2, space="PSUM") as mm1ps,
                    tc.tile_pool(name="mm2ps", bufs=1, space="PSUM") as mm2ps,
                    tc.tile_pool(name="w2p", bufs=6) as w2p,
                    tc.tile_pool(name="yout", bufs=3) as yout,
                ):
                    w2t_q = []

                    def w2_load(half, kt):
                        w2t = w2p.tile([P, 512], BF, tag="w2t")
                        nc.gpsimd.dma_start(w2t, w2_d[half, kt])
                        w2t_q.append(w2t)

                    for kt in range(3):
                        w2_load(0, kt)
                    ps2 = {t: mm2ps.tile([P, 512], F32, name=f"y2a_{t}",
                                         tag=f"y2_{t}") for t in range(TT)}
                    for ft in range(NFT):
                        ps = mm1ps.tile([P, TPC], F32, tag="mm1")
                        for kt in range(KT_E):
                            nc.tensor.matmul(
                                ps, w1_sb[kt][:, ft * P : (ft + 1) * P],
                                z2T_sb[:, :, kt, :],
                                start=(kt == 0), stop=(kt == KT_E - 1))
                        nc.scalar.activation(fT_t[ft], ps, Act.Relu,
                                             bias=b1_sb[:, ft : ft + 1], scale=1.0)
                        if ft + 3 < NFT:
                            w2_load(0, ft + 3)
                        w2t = w2t_q.pop(0)
                        for t in range(TT):
                            nc.tensor.matmul(
                                ps2[t], fT_t[ft][:, t * P : (t + 1) * P],
                                w2t, start=(ft == 0), stop=False)

                    for t in range(TT):
                        ns = slice(0, 512)
                        nc.tensor.matmul(ps2[t], ones_sb, b2_sb[0:1, ns],
                                         start=False, stop=True)
                        yt = yout.tile([P, 512], F32, tag="yt")
                        nc.vector.tensor_add(yt, ps2[t], x2_t[t][:, ns])
                        nc.sync.dma_start(y_view[:, t, ns], yt)

                    for kt in range(3):
                        w2_load(1, kt)
                    ps2b = {t: mm2ps.tile([P, 512], F32, name=f"y2b_{t}",
                                          tag=f"y2_{t}") for t in range(TT)}
                    for kt in range(KT_F):
                        if kt + 3 < KT_F:
                            w2_load(1, kt + 3)
                        w2t = w2t_q.pop(0)
                        for t in range(TT):
                            nc.tensor.matmul(
                                ps2b[t], fT_t[kt][:, t * P : (t + 1) * P],
                                w2t, start=(kt == 0), stop=False)
                    for t in range(TT):
                        ns = slice(512, 1024)
                        nc.tensor.matmul(ps2b[t], ones_sb, b2_sb[0:1, ns],
                                         start=False, stop=True)
                        yt = yout.tile([P, 512], F32, tag="yt")
                        nc.vector.tensor_add(yt, ps2b[t], x2_t[t][:, ns])
                        nc.sync.dma_start(y_view[:, t, ns], yt)

    nc.compile()
    return nc


_NC_CACHE = {}


def _get_nc(C):
    if C not in _NC_CACHE:
        _NC_CACHE[C] = _build(C)
    return _NC_CACHE[C]


def make_in_maps(inputs, C):
    """Host-side sharding + LN-gain/bias folding. Values are numpy fp32."""
    TPC = C // NCORES
    KTE = E // P
    x32 = inputs["x"].reshape(C, E).astype(np.float32)
    x = np.ascontiguousarray(x32.astype(bf16))
    Wq, Wk, Wv = inputs["Wq"], inputs["Wk"], inputs["Wv"]
    Wo, bo = inputs["Wo"], inputs["bo"]
    W1, b1, W2, b2 = inputs["W1"], inputs["b1"], inputs["W2"], inputs["b2"]
    g1, bl1 = inputs["ln1_g"].astype(np.float64), inputs["ln1_b"].astype(np.float64)
    g2, bl2 = inputs["ln2_g"].astype(np.float64), inputs["ln2_b"].astype(np.float64)

    # v bias folds through Wo into bo (softmax weights sum to 1)
    bv_all = np.concatenate([bl1 @ Wv[h].astype(np.float64) for h in range(NH)])
    bo_eff = (bo.astype(np.float64) + bv_all @ Wo.astype(np.float64)).astype(
        np.float32)

    wo_h = np.ascontiguousarray(Wo.reshape(KTE, P, D).astype(bf16))
    w1_h = np.ascontiguousarray(
        (g2[:, None] * W1.astype(np.float64)).astype(np.float32)
        .reshape(KTE, P, FF).astype(bf16))
    b1_eff = (b1.astype(np.float64) + bl2 @ W1.astype(np.float64)).astype(
        np.float32)
    b1c = np.ascontiguousarray(b1_eff.reshape(FF // P, P).T)  # (P, NFT)
    w2_h = np.ascontiguousarray(
        W2.reshape(FF // P, P, 2, E // 2).transpose(2, 0, 1, 3).astype(bf16))
    b2r = np.ascontiguousarray(b2.reshape(1, E).astype(np.float32).astype(bf16))
    boer = np.ascontiguousarray(bo_eff.reshape(1, D).astype(bf16))
    tri = np.triu(np.ones((P, P), np.float32))
    maskz = np.zeros((4, P, 512), np.float32)
    for dd in range(4):
        maskz[dd, :, dd * P : (dd + 1) * P] = tri
        maskz[dd, :, (dd + 1) * P :] = 1.0
    maskz2 = np.ascontiguousarray(
        np.concatenate([maskz, maskz], axis=2).astype(bf16))
    maskz = np.ascontiguousarray(maskz.astype(bf16))
    # bc-broadcast selectors: den row r (=source core kt) -> that head's dims
    sel = np.zeros((2, KTE, 8, HS), np.float32)
    for h in range(2):
        for kt in range(KTE):
            sel[h, kt, kt, :] = 1.0
    sel = np.ascontiguousarray(sel.astype(bf16))

    in_maps = []
    for i in range(NCORES):
        h0, h1 = HPC * i, HPC * i + 1
        wq_eff = np.concatenate(
            [(g1[:, None] * Wq[h].astype(np.float64)) for h in (h0, h1)], axis=1
        ).astype(np.float32)  # (E, 128)
        wk_eff = np.concatenate(
            [(g1[:, None] * Wk[h].astype(np.float64)) for h in (h0, h1)], axis=1
        ).astype(np.float32)
        wv_eff = np.concatenate(
            [(g1[:, None] * Wv[h].astype(np.float64)) for h in (h0, h1)], axis=1
        ).astype(np.float32)
        bq = np.concatenate(
            [bl1 @ Wq[h].astype(np.float64) for h in (h0, h1)]).astype(np.float32)
        in_maps.append(
            {
                "x": x,
                "xs": np.ascontiguousarray(x32[i * TPC : (i + 1) * TPC]),
                "wq": np.ascontiguousarray(wq_eff.reshape(KTE, P, P).astype(bf16)),
                "wk": np.ascontiguousarray(wk_eff.reshape(KTE, P, P).astype(bf16)),
                "wv": np.ascontiguousarray(wv_eff.reshape(KTE, P, P).astype(bf16)),
                "bq": np.ascontiguousarray(bq.reshape(P, 1)),
                "wo": wo_h,
                "boe": boer,
                "w1": w1_h,
                "b1c": b1c,
                "w2": w2_h,
                "b2_r": b2r,
                "maskz": maskz,
                "maskz2": maskz2,
                "sel": sel,
            }
        )
    return in_maps


def run(inputs, C=4096, trace=False):
    nc = _get_nc(C)
    in_maps = make_in_maps(inputs, C)
    res = run_bass_kernel_spmd(nc, in_maps, core_ids=list(range(NCORES)), trace=trace)
    TPC = C // NCORES
    y = np.concatenate(
        [np.asarray(res.results[i]["y"]).reshape(TPC, E) for i in range(NCORES)], 0
    )
    return y.reshape(1, C, E).astype(np.float32), res


def kernel(**inputs):
    inputs = {k: np.asarray(v) for k, v in inputs.items()}
    y, _ = run(inputs, C=4096, trace=False)
    return y
